# revision 18
# baseline (speedup 1.0000x reference)
"""Multi-head attention (B=2, S=2048, D=1024, H=16, dk=64) on 8 TRN2 cores.

Sharding: core c handles batch b = c//4 and head group hg = c%4 (4 heads,
256 head-dims).  Each core computes Q/K/V projections for its head slice,
attention for its 4 heads, and a partial output projection against the
matching 256-row slice of Wo.  The host sums the 4 partials per batch.

Key optimizations over the fp32 baseline:
  - All matmuls in bf16 (fp32 streams at 1/4 rate on the PE).
  - Host-side key compaction: the key mask is position-only, so masked
    keys/values are gathered out on the host (~2048 -> ~1024+pad).  Padded
    key slots get bias -300 so exp() underflows to exactly 0.
  - Software-pipelined emission: the whole attention phase (4 blocks x nt
    key-tiles x 2 query-chunks) is one flat step pipeline.  Scores+exp for
    step s+2 are emitted BEFORE the PV/denominator matmuls of step s, so
    the PE computes the next tile's scores while the Scalar engine's exp
    of the current tile runs -- both engines stay continuously busy and the
    Scalar exp stream never waits (it is the throughput floor).
  - QKV projection producers are emitted as per-step fillers inside the
    first block (key-proj in 128-key chunks) instead of up front, so the
    first exp starts after ~3 MB of DMA instead of ~8.4 MB.
  - Scores staged per k-tile in [128, 1024] two-bank PSUM tiles so each
    ACT exp instruction covers 1024 elements/partition.  ACT does exp ONLY.
  - Block boundaries overlap: the next block's first scores/exp are
    emitted before the previous block's normalize chain, which uses
    reciprocal_approx_fast (~5x cheaper than bit-exact reciprocal).
  - The tail normalize is query-chunk-major and the final output
    projection is interleaved per chunk to shorten the epilogue.
  - bk/bv dropped on device (softmax shift invariance / attn rows sum to
    1); host adds the constant correction row bv@Wo.T + bo.

Math (exact up to rounding):
  scores^T[k,q] = K.Q^T computed transposed so the key mask is a
  per-partition bias folded into the Exp activation, and P^T feeds the PV
  matmul directly; ctx = (P^T)^T V / denom with denom broadcast across
  partitions via a PE rank-1 matmul.
"""

import numpy as np
import ml_dtypes

from contextlib import ExitStack

import concourse.bass as bass
import concourse.mybir as mybir
import concourse.tile as tile
from concourse import bacc
from concourse.bass_utils import run_bass_kernel_spmd

F32 = mybir.dt.float32
BF16 = mybir.dt.bfloat16
NPBF16 = ml_dtypes.bfloat16

D_MODEL = 1024
S = 2048
BATCH = 2
N_CORES = 8
DK = 64
DO = 256  # 4 heads * 64 dims per core
MASK_BIAS = -300.0

AF = mybir.ActivationFunctionType
ALU = mybir.AluOpType


def build_program(nt: int) -> bass.Bass:
    """nt = number of 128-wide key tiles after compaction."""
    nk = nt * 128
    nc = bacc.Bacc("TRN2", target_bir_lowering=False, debug=False,
                   num_devices=N_CORES)

    # x inputs are host-arranged so each DMA chunk is one contiguous run
    # per partition: xq [p, qc, t, 512], xk/xv [p, kt, t, 128]
    xqT = nc.declare_dram_parameter("xqT", [128, 4, 8, 512], BF16,
                                    isOutput=False)
    xkT = nc.declare_dram_parameter("xkT", [128, nt, 8, 128], BF16,
                                    isOutput=False)
    xvT = nc.declare_dram_parameter("xvT", [128, nt, 8, 128], BF16,
                                    isOutput=False)
    wqT = nc.declare_dram_parameter("wqT", [128, 8, DO], BF16, isOutput=False)
    wkT = nc.declare_dram_parameter("wkT", [128, 8, DO], BF16, isOutput=False)
    wvT = nc.declare_dram_parameter("wvT", [128, 8, DO], BF16, isOutput=False)
    woT = nc.declare_dram_parameter("woT", [128, 2, D_MODEL], BF16,
                                    isOutput=False)
    bq2 = nc.declare_dram_parameter("bq2", [128, 2], F32, isOutput=False)
    maskb = nc.declare_dram_parameter("maskb", [128, nt], F32, isOutput=False)
    out = nc.declare_dram_parameter("out", [S, D_MODEL], BF16, isOutput=True)

    with tile.TileContext(nc) as tc, ExitStack() as ctx:
        consts = ctx.enter_context(tc.tile_pool(name="consts", bufs=1))
        big = ctx.enter_context(tc.tile_pool(name="big", bufs=1))
        xpool = ctx.enter_context(tc.tile_pool(name="xpool", bufs=2))
        xvpool = ctx.enter_context(tc.tile_pool(name="xvpool", bufs=2))
        ptpool = ctx.enter_context(tc.tile_pool(name="ptpool", bufs=4))
        rpool = ctx.enter_context(tc.tile_pool(name="rpool", bufs=2))
        opool = ctx.enter_context(tc.tile_pool(name="opool", bufs=2))
        # PSUM: sc 2x[128,1024]=4 banks, ctx 2x[128,512]=2, dn 1, mm 1 -> 8
        ps_sc = ctx.enter_context(tc.tile_pool(name="ps_sc", bufs=2, space="PSUM"))
        ps_ctx = ctx.enter_context(tc.tile_pool(name="ps_ctx", bufs=2, space="PSUM"))
        ps_dn = ctx.enter_context(tc.tile_pool(name="ps_dn", bufs=1, space="PSUM"))
        ps_mm = ctx.enter_context(tc.tile_pool(name="ps_mm", bufs=1, space="PSUM"))

        # ---- constants / weights in SBUF (prefix DMAs only; wv/wo later) ----
        bq_sb = consts.tile([128, 2], F32)
        nc.sync.dma_start(bq_sb, bq2[:, :])
        mask_sb = consts.tile([128, nt], F32)
        nc.sync.dma_start(mask_sb, maskb[:, :])
        ones_bc = consts.tile([128, 64], BF16)
        nc.vector.memset(ones_bc[:, :], 1.0)
        zeros_sb = consts.tile([128, 512], BF16)
        nc.vector.memset(zeros_sb[:, :], 0.0)
        wq_sb = consts.tile([128, 8, DO], BF16)
        nc.sync.dma_start(wq_sb, wqT[:, :, :])
        wk_sb = consts.tile([128, 8, DO], BF16)
        nc.sync.dma_start(wk_sb, wkT[:, :, :])
        wv_sb = consts.tile([128, 8, DO], BF16)
        wo_sb = consts.tile([128, 2, D_MODEL], BF16)

        # ---- persistent activations ----
        qT_sb = big.tile([128, 2, S], BF16)     # Q^T: (head-pair dims, hp, q)
        kT_sb = big.tile([128, 2, nk], BF16)    # K^T: (head-pair dims, hp, k)
        ctx_pair = big.tile([128, 2, S], BF16)  # (pair dim c, hp, q), normalized
        v_tiles = [big.tile([128, 4, DK], BF16, name=f"v{st}", tag=f"v{st}")
                   for st in range(nt)]

        def v_proj(st):
            xv_t = xvpool.tile([128, 8, 128], BF16, name="xv_t", tag="xv")
            nc.sync.dma_start(xv_t, xvT[:, st, :, :])
            ps = ps_mm.tile([128, DO], F32, name="ps_v", tag="mm")
            for di in range(8):
                nc.tensor.matmul(ps, lhsT=xv_t[:, di, :], rhs=wv_sb[:, di, :],
                                 start=(di == 0), stop=(di == 7))
            nc.vector.tensor_copy(
                out=v_tiles[st][:, :, :],
                in_=ps.rearrange("p (h d) -> p h d", h=4))

        def k_proj(coff, cw, dn_free=False):
            # one x-chunk DMA feeds both head-pairs (dt); the second head
            # pair may borrow the dn bank only outside accumulation windows
            xk_t = xpool.tile([128, 8, 128], BF16, name="xk_t", tag="xk")
            nc.sync.dma_start(xk_t, xkT[:, coff // 128, :, :])
            second = (ps_dn, "dn") if dn_free else (ps_mm, "mm")
            for dt_, pool, tag in ((0, ps_mm, "mm"), (1, *second)):
                ps = pool.tile([128, 512], F32, name="ps_k", tag=tag)
                for di in range(8):
                    nc.tensor.matmul(
                        ps[:, 0:cw],
                        lhsT=wk_sb[:, di, dt_ * 128:(dt_ + 1) * 128],
                        rhs=xk_t[:, di, 0:cw], start=(di == 0), stop=(di == 7))
                nc.vector.tensor_copy(out=kT_sb[:, dt_, coff:coff + cw],
                                      in_=ps[:, 0:cw])

        def q_proj(sc, dn_free=False):
            xq_t = xpool.tile([128, 8, 512], BF16, name="xq_t", tag="x")
            nc.sync.dma_start(xq_t, xqT[:, sc, :, :])
            second = (ps_dn, "dn") if dn_free else (ps_mm, "mm")
            for dt_, pool, tag in ((0, ps_mm, "mm"), (1, *second)):
                ps = pool.tile([128, 512], F32, name="ps_q", tag=tag)
                for di in range(8):
                    nc.tensor.matmul(
                        ps, lhsT=wq_sb[:, di, dt_ * 128:(dt_ + 1) * 128],
                        rhs=xq_t[:, di, :], start=(di == 0), stop=(di == 7))
                nc.vector.tensor_scalar(
                    out=qT_sb[:, dt_, sc * 512:(sc + 1) * 512], in0=ps,
                    scalar1=bq_sb[:, dt_:dt_ + 1], scalar2=None, op0=ALU.add)

        def out_proj(so, pools, use_act=False):
            o_sb = opool.tile([128, D_MODEL], BF16, name="o_sb", tag="o")
            for oc in range(2):
                pool, tag = pools[(2 * so + oc) % len(pools)]
                ps = pool.tile([128, 512], F32, name="ps_o", tag=tag)
                for hp in range(2):
                    nc.tensor.matmul(
                        ps, lhsT=ctx_pair[:, hp, so * 128:(so + 1) * 128],
                        rhs=wo_sb[:, hp, oc * 512:(oc + 1) * 512],
                        start=(hp == 0), stop=(hp == 1))
                dst = o_sb[:, oc * 512:(oc + 1) * 512]
                if use_act and oc == 1:
                    nc.scalar.copy(dst, ps)
                else:
                    nc.vector.tensor_copy(out=dst, in_=ps)
            nc.sync.dma_start(out[so * 128:(so + 1) * 128, :], o_sb)

        # ---- attention step pipeline over all 4 (hp, qh) blocks ----
        blocks = [(0, 0), (1, 0), (0, 1), (1, 1)]
        steps = [(bi, st, qc)
                 for bi in range(4) for st in range(nt) for qc in (0, 1)]
        # denominator col-tile positions: (hh, qc) -> partition 32*j
        dnj = {(0, 0): 2, (1, 0): 3, (0, 1): 0, (1, 1): 1}
        state = {}
        pts = {}

        def bank_opens(bi):
            # Zero-weight full-bank matmuls open each accumulation bank:
            # clears has_written across all 128 partitions so the real
            # streams (start=False) overwrite-on-first-touch per element.
            st_ = state[bi] = {
                "ctx": [ps_ctx.tile([128, 512], F32, name=f"ctx{hh}",
                                    tag="ctx") for hh in range(2)],
                "dn": ps_dn.tile([128, 512], F32, name="dn", tag="dn"),
            }
            for bank in (st_["ctx"][0], st_["ctx"][1], st_["dn"]):
                nc.tensor.matmul(bank, lhsT=zeros_sb[:, 0:128], rhs=zeros_sb,
                                 start=True, stop=False, skip_group_check=True)

        def prepare(s):
            bi, st, qc = s
            hp, qh = blocks[bi]
            qoff = qh * 1024 + qc * 512
            sps = ps_sc.tile([128, 1024], F32, name="sps", tag="sc")
            for hh in range(2):
                nc.tensor.matmul(
                    sps[:, hh * 512:(hh + 1) * 512],
                    lhsT=kT_sb[64 * hh:64 * (hh + 1), hp,
                               st * 128:(st + 1) * 128],
                    rhs=qT_sb[64 * hh:64 * (hh + 1), hp, qoff:qoff + 512],
                    start=True, stop=True, tile_position=(64 * hh, 0))
            pt = ptpool.tile([128, 1024], BF16, name="pt", tag="pt")
            nc.scalar.activation(out=pt, in_=sps, func=AF.Exp,
                                 bias=mask_sb[:, st:st + 1], scale=0.125)
            pts[s] = pt

        def pv_dn(s):
            bi, st, qc = s
            hp, qh = blocks[bi]
            pt = pts.pop(s)
            ctx_ps = state[bi]["ctx"]
            dn_ps = state[bi]["dn"]
            for hh in range(2):
                nc.tensor.matmul(
                    ctx_ps[hh][64 * qc:64 * (qc + 1), :],
                    lhsT=v_tiles[st][:, 2 * hp + hh, :],
                    rhs=pt[:, hh * 512:(hh + 1) * 512],
                    start=(st == 0 and qc == 0),
                    stop=(st == nt - 1),
                    tile_position=(0, 64 * qc),
                    skip_group_check=True)
            for hh in range(2):
                j = dnj[(hh, qc)]
                nc.tensor.matmul(
                    dn_ps[32 * j:32 * j + 1, :],
                    lhsT=ones_bc[:, j:j + 1],
                    rhs=pt[:, hh * 512:(hh + 1) * 512],
                    start=(st == 0 and qc == 0 and hh == 0),
                    stop=(st == nt - 1),
                    tile_position=(0, 32 * j),
                    skip_group_check=True)

        def normalize_qc(bi, qc, cb=None):
            # per query-chunk normalize: reciprocal of this chunk's two
            # denominator rows, PE-broadcast to 64 partitions per head,
            # multiply into ctx_pair.  qc0 rows live at partitions 64-127
            # of the dn bank (j=2,3), qc1 at 0-63 (j=0,1).
            hp, qh = blocks[bi]
            ctx_ps = state[bi]["ctx"]
            dn_ps = state[bi]["dn"]
            # full-bank reciprocal (partition offset 0 required); rows of
            # the other chunk may hold partial sums nothing reads
            rp = rpool.tile([128, 512], F32, name="rp", tag="rp")
            nc.vector.reciprocal_approx_fast(out=rp, in_=dn_ps)
            rpb = rpool.tile([65, 512], BF16, name="rpb", tag="rpb")
            # hh0 -> partition 0, hh1 -> partition 64
            j0, j1 = dnj[(0, qc)], dnj[(1, qc)]
            nc.vector.tensor_copy(out=rpb[0:1, :],
                                  in_=rp[32 * j0:32 * j0 + 1, :])
            nc.vector.tensor_copy(out=rpb[64:65, :],
                                  in_=rp[32 * j1:32 * j1 + 1, :])
            r_ps = ps_mm.tile([128, 512], F32, name="r_ps", tag="mm")
            nc.tensor.matmul(r_ps[0:64, :], lhsT=ones_bc[0:1, :],
                             rhs=rpb[0:1, :], start=True, stop=True,
                             tile_position=(0, 0))
            nc.tensor.matmul(r_ps[64:128, :], lhsT=ones_bc[64:65, :],
                             rhs=rpb[64:65, :], start=True, stop=True,
                             tile_position=(64, 64))
            r_sb = rpool.tile([128, 512], F32, name="r_sb", tag="r_sb")
            nc.vector.tensor_copy(out=r_sb, in_=r_ps)
            qoff = qh * 1024 + qc * 512
            for hh in range(2):
                nc.vector.tensor_tensor(
                    ctx_pair[64 * hh:64 * (hh + 1), hp, qoff:qoff + 512],
                    ctx_ps[hh][64 * qc:64 * (qc + 1), :],
                    r_sb[64 * hh:64 * (hh + 1), :],
                    ALU.mult)
            if cb is not None:
                cb(qc)

        # ---- producer fillers: (step index) -> [thunks] ----
        # kT tile st must be resident before prepare((0,st,0)) fires at step
        # 2*st-3; v_tiles[st] before pv_dn((0,st,*)) at step 2*st.
        fillers = {}

        def add_filler(i, f):
            fillers.setdefault(i, []).append(f)

        add_filler(0, lambda: q_proj(1))
        for st in range(1, nt):
            add_filler(max(2 * st - 3, 0),
                       lambda st=st: k_proj(st * 128, 128))
            if st >= 1:
                add_filler(max(2 * st - 1, 1), lambda st=st: v_proj(st))
        add_filler(2 * nt - 1, lambda: nc.sync.dma_start(
            wo_sb, woT[:, :, :]))
        add_filler(2 * nt + 1, lambda: q_proj(2))
        add_filler(2 * nt + 3, lambda: q_proj(3))
        # delayed past the previous block's normalize chain so the PSUM
        # allocation doesn't head-of-line-block the PE queue
        for k in range(4):
            add_filler(4 * nt + 5 + 2 * k,
                       lambda k=k: out_proj(k, [(ps_mm, "mm")]))
            add_filler(6 * nt + 5 + 2 * k,
                       lambda k=k: out_proj(4 + k, [(ps_mm, "mm")]))

        def tail_cb(qc):
            for so in range(8 + 4 * qc, 12 + 4 * qc):
                pools = [(ps_mm, "mm"), (ps_dn, "dn")]
                if qc == 1:
                    pools.append((ps_ctx, "ctx"))
                out_proj(so, pools, use_act=True)

        # ---- prefix: HAM warmup, then minimal producers for (0,0,0) ----
        wm = ps_mm.tile([128, 512], F32, name="wm", tag="mm")
        for w in range(12):
            nc.tensor.matmul(wm, lhsT=zeros_sb[:, 0:128], rhs=zeros_sb,
                             start=(w == 0), stop=(w == 11))
        k_proj(0, 128, dn_free=True)
        q_proj(0, dn_free=True)
        nc.sync.dma_start(wv_sb, wvT[:, :, :])
        v_proj(0)
        bank_opens(0)

        # ---- the pipeline ----
        prepared = 0
        for i, s in enumerate(steps):
            for f in fillers.get(i, ()):
                f()
            while prepared < min(i + 3, len(steps)):
                prepare(steps[prepared])
                prepared += 1
            pv_dn(s)
            bi, st, qc = s
            if st == nt - 1:  # last two steps of block bi
                if bi < 3:
                    if qc == 1:
                        normalize_qc(bi, 0)
                        normalize_qc(bi, 1)
                        bank_opens(bi + 1)
                else:
                    normalize_qc(bi, qc, cb=tail_cb)

    nc.finalize()
    return nc


_NC_CACHE: dict = {}
LAST_RESULTS = None


def _get_program(nt: int) -> bass.Bass:
    if nt not in _NC_CACHE:
        _NC_CACHE[nt] = build_program(nt)
    return _NC_CACHE[nt]


def make_in_maps(query, key_, value, mask, Wq, bq, Wk, Wv, Wo, nt):
    nk = nt * 128
    in_maps = []
    idxs = [np.flatnonzero(mask[b, 0, 0]) for b in range(BATCH)]
    for b in range(BATCH):
        n = len(idxs[b])
        xk = np.zeros((nk, D_MODEL), np.float32)
        xv = np.zeros((nk, D_MODEL), np.float32)
        xk[:n] = key_[b][idxs[b]]
        xv[:n] = value[b][idxs[b]]
        mbf = np.full(nt * 128, np.float32(MASK_BIAS), np.float32)
        mbf[:n] = 0.0
        mb = np.ascontiguousarray(mbf.reshape(nt, 128).T)
        # device layouts: xq [p, qc, t, 512], xk/xv [p, kt, t, 128],
        # w [p, t, 256], wo [p, hp, 1024] -- one contiguous DMA run per
        # partition per chunk
        xqTb = np.ascontiguousarray(
            query[b].T.astype(NPBF16).reshape(8, 128, 4, 512)
            .transpose(1, 2, 0, 3))
        xkTb = np.ascontiguousarray(
            xk.T.astype(NPBF16).reshape(8, 128, nt, 128)
            .transpose(1, 2, 0, 3))
        xvTb = np.ascontiguousarray(
            xv.T.astype(NPBF16).reshape(8, 128, nt, 128)
            .transpose(1, 2, 0, 3))
        for hg in range(4):
            sl = slice(hg * DO, (hg + 1) * DO)
            in_maps.append({
                "xqT": xqTb,
                "xkT": xkTb,
                "xvT": xvTb,
                "wqT": np.ascontiguousarray(
                    Wq[sl, :].T.astype(NPBF16).reshape(8, 128, DO)
                    .transpose(1, 0, 2)),
                "wkT": np.ascontiguousarray(
                    Wk[sl, :].T.astype(NPBF16).reshape(8, 128, DO)
                    .transpose(1, 0, 2)),
                "wvT": np.ascontiguousarray(
                    Wv[sl, :].T.astype(NPBF16).reshape(8, 128, DO)
                    .transpose(1, 0, 2)),
                "woT": np.ascontiguousarray(
                    Wo[:, sl].T.astype(NPBF16).reshape(2, 128, D_MODEL)
                    .transpose(1, 0, 2)),
                "bq2": np.ascontiguousarray(
                    bq[sl].reshape(2, 128).T.astype(np.float32)),
                "maskb": mb,
            })
    # reorder: core c = b*4 + hg
    return in_maps


def kernel(query, key_, value, mask, Wq, bq, Wk, bk, Wv, bv, Wo, bo):
    global LAST_RESULTS
    query = np.asarray(query, dtype=np.float32)
    key_ = np.asarray(key_, dtype=np.float32)
    value = np.asarray(value, dtype=np.float32)
    mask = np.asarray(mask)
    counts = [int(mask[b, 0, 0].sum()) for b in range(BATCH)]
    nt = max((max(counts) + 127) // 128, 1)
    nc = _get_program(nt)
    in_maps = make_in_maps(query, key_, value, mask,
                           np.asarray(Wq), np.asarray(bq), np.asarray(Wk),
                           np.asarray(Wv), np.asarray(Wo), nt)
    res = run_bass_kernel_spmd(nc, in_maps, list(range(N_CORES)))
    LAST_RESULTS = res
    corr = (np.asarray(bv, dtype=np.float32) @ np.asarray(Wo, dtype=np.float32).T
            + np.asarray(bo, dtype=np.float32))
    out = np.zeros((BATCH, S, D_MODEL), np.float32)
    for c in range(N_CORES):
        out[c // 4] += np.asarray(res.results[c]["out"], dtype=np.float32)
    out += corr[None, None, :]
    return out


# revision 20
# speedup vs baseline: 1.0773x; 1.0773x over previous
"""Multi-head attention (B=2, S=2048, D=1024, H=16, dk=64) on 8 TRN2 cores.

Sharding: core c handles batch b = c//4 and head group hg = c%4 (4 heads,
256 head-dims).  Each core computes Q/K/V projections for its head slice,
attention for its 4 heads, and a partial output projection against the
matching 256-row slice of Wo.  The host sums the 4 partials per batch.

Key optimizations over the fp32 baseline:
  - All matmuls in bf16 (fp32 streams at 1/4 rate on the PE).
  - Host-side key compaction: the key mask is position-only, so masked
    keys/values are gathered out on the host (~2048 -> ~1024+pad).  Padded
    key slots get bias -300 so exp() underflows to exactly 0.
  - Software-pipelined emission: the whole attention phase (4 blocks x nt
    key-tiles x 2 query-chunks) is one flat step pipeline.  Scores+exp for
    step s+2 are emitted BEFORE the PV/denominator matmuls of step s, so
    the PE computes the next tile's scores while the Scalar engine's exp
    of the current tile runs -- both engines stay continuously busy and the
    Scalar exp stream never waits (it is the throughput floor).
  - QKV projection producers are emitted as per-step fillers inside the
    first block (key-proj in 128-key chunks) instead of up front, so the
    first exp starts after ~3 MB of DMA instead of ~8.4 MB.
  - Scores staged per k-tile in [128, 1024] two-bank PSUM tiles so each
    ACT exp instruction covers 1024 elements/partition.  ACT does exp ONLY.
  - Block boundaries overlap: the next block's first scores/exp are
    emitted before the previous block's normalize chain, which uses
    reciprocal_approx_fast (~5x cheaper than bit-exact reciprocal).
  - The tail normalize is query-chunk-major and the final output
    projection is interleaved per chunk to shorten the epilogue.
  - bk/bv dropped on device (softmax shift invariance / attn rows sum to
    1); host adds the constant correction row bv@Wo.T + bo.

Math (exact up to rounding):
  scores^T[k,q] = K.Q^T computed transposed so the key mask is a
  per-partition bias folded into the Exp activation, and P^T feeds the PV
  matmul directly; ctx = (P^T)^T V / denom with denom broadcast across
  partitions via a PE rank-1 matmul.
"""

import numpy as np
import ml_dtypes

from contextlib import ExitStack

import concourse.bass as bass
import concourse.mybir as mybir
import concourse.tile as tile
from concourse import bacc
from concourse.bass_utils import run_bass_kernel_spmd

F32 = mybir.dt.float32
BF16 = mybir.dt.bfloat16
NPBF16 = ml_dtypes.bfloat16

D_MODEL = 1024
S = 2048
BATCH = 2
N_CORES = 8
DK = 64
DO = 256  # 4 heads * 64 dims per core
MASK_BIAS = -300.0

AF = mybir.ActivationFunctionType
ALU = mybir.AluOpType


def build_program(nt: int) -> bass.Bass:
    """nt = number of 128-wide key tiles after compaction."""
    nk = nt * 128
    nc = bacc.Bacc("TRN2", target_bir_lowering=False, debug=False,
                   num_devices=N_CORES)

    # x inputs are host-arranged so each DMA chunk is one contiguous run
    # per partition: xq [p, qc, t, 512], xk/xv [p, kt, t, 128]
    xqT = nc.declare_dram_parameter("xqT", [128, 4, 8, 512], BF16,
                                    isOutput=False)
    xkT = nc.declare_dram_parameter("xkT", [128, nt, 8, 128], BF16,
                                    isOutput=False)
    xvT = nc.declare_dram_parameter("xvT", [128, nt, 8, 128], BF16,
                                    isOutput=False)
    wqT = nc.declare_dram_parameter("wqT", [128, 8, DO], BF16, isOutput=False)
    wkT = nc.declare_dram_parameter("wkT", [128, 8, DO], BF16, isOutput=False)
    wvT = nc.declare_dram_parameter("wvT", [128, 8, DO], BF16, isOutput=False)
    woT = nc.declare_dram_parameter("woT", [128, 2, D_MODEL], BF16,
                                    isOutput=False)
    bq2 = nc.declare_dram_parameter("bq2", [128, 2], F32, isOutput=False)
    maskb = nc.declare_dram_parameter("maskb", [128, nt], F32, isOutput=False)
    out = nc.declare_dram_parameter("out", [S, D_MODEL], BF16, isOutput=True)

    with tile.TileContext(nc) as tc, ExitStack() as ctx:
        consts = ctx.enter_context(tc.tile_pool(name="consts", bufs=1))
        big = ctx.enter_context(tc.tile_pool(name="big", bufs=1))
        xpool = ctx.enter_context(tc.tile_pool(name="xpool", bufs=2))
        xvpool = ctx.enter_context(tc.tile_pool(name="xvpool", bufs=2))
        ptpool = ctx.enter_context(tc.tile_pool(name="ptpool", bufs=4))
        rpool = ctx.enter_context(tc.tile_pool(name="rpool", bufs=2))
        opool = ctx.enter_context(tc.tile_pool(name="opool", bufs=2))
        # PSUM: sc 2x[128,1024]=4 banks, ctx 2x[128,512]=2, dn 1, mm 1 -> 8
        ps_sc = ctx.enter_context(tc.tile_pool(name="ps_sc", bufs=2, space="PSUM"))
        ps_ctx = ctx.enter_context(tc.tile_pool(name="ps_ctx", bufs=2, space="PSUM"))
        ps_dn = ctx.enter_context(tc.tile_pool(name="ps_dn", bufs=1, space="PSUM"))
        ps_mm = ctx.enter_context(tc.tile_pool(name="ps_mm", bufs=1, space="PSUM"))

        # ---- constants / weights in SBUF (prefix DMAs only; wv/wo later) ----
        bq_sb = consts.tile([128, 2], F32)
        nc.sync.dma_start(bq_sb, bq2[:, :])
        mask_sb = consts.tile([128, nt], F32)
        nc.sync.dma_start(mask_sb, maskb[:, :])
        ones_bc = consts.tile([128, 64], BF16)
        nc.vector.memset(ones_bc[:, :], 1.0)
        zeros_sb = consts.tile([128, 512], BF16)
        nc.vector.memset(zeros_sb[:, :], 0.0)
        wq_sb = consts.tile([128, 8, DO], BF16)
        nc.sync.dma_start(wq_sb, wqT[:, :, :])
        wk_sb = consts.tile([128, 8, DO], BF16)
        nc.sync.dma_start(wk_sb, wkT[:, :, :])
        wv_sb = consts.tile([128, 8, DO], BF16)
        wo_sb = consts.tile([128, 2, D_MODEL], BF16)

        # ---- persistent activations ----
        qT_sb = big.tile([128, 2, S], BF16)     # Q^T: (head-pair dims, hp, q)
        kT_sb = big.tile([128, 2, nk], BF16)    # K^T: (head-pair dims, hp, k)
        ctx_pair = big.tile([128, 2, S], BF16)  # (pair dim c, hp, q), normalized
        v_tiles = [big.tile([128, 4, DK], BF16, name=f"v{st}", tag=f"v{st}")
                   for st in range(nt)]

        def v_proj(st):
            xv_t = xvpool.tile([128, 8, 128], BF16, name="xv_t", tag="xv")
            nc.sync.dma_start(xv_t, xvT[:, st, :, :])
            ps = ps_mm.tile([128, DO], F32, name="ps_v", tag="mm")
            for di in range(8):
                nc.tensor.matmul(ps, lhsT=xv_t[:, di, :], rhs=wv_sb[:, di, :],
                                 start=(di == 0), stop=(di == 7))
            nc.vector.tensor_copy(
                out=v_tiles[st][:, :, :],
                in_=ps.rearrange("p (h d) -> p h d", h=4))

        def k_proj(coff, cw, dn_free=False):
            # one x-chunk DMA feeds both head-pairs (dt); the second head
            # pair may borrow the dn bank only outside accumulation windows
            xk_t = xpool.tile([128, 8, 128], BF16, name="xk_t", tag="xk")
            nc.sync.dma_start(xk_t, xkT[:, coff // 128, :, :])
            second = (ps_dn, "dn") if dn_free else (ps_mm, "mm")
            for dt_, pool, tag in ((0, ps_mm, "mm"), (1, *second)):
                ps = pool.tile([128, 512], F32, name="ps_k", tag=tag)
                for di in range(8):
                    nc.tensor.matmul(
                        ps[:, 0:cw],
                        lhsT=wk_sb[:, di, dt_ * 128:(dt_ + 1) * 128],
                        rhs=xk_t[:, di, 0:cw], start=(di == 0), stop=(di == 7))
                nc.vector.tensor_copy(out=kT_sb[:, dt_, coff:coff + cw],
                                      in_=ps[:, 0:cw])

        def q_proj(sc, dn_free=False):
            xq_t = xpool.tile([128, 8, 512], BF16, name="xq_t", tag="x")
            nc.sync.dma_start(xq_t, xqT[:, sc, :, :])
            second = (ps_dn, "dn") if dn_free else (ps_mm, "mm")
            for dt_, pool, tag in ((0, ps_mm, "mm"), (1, *second)):
                ps = pool.tile([128, 512], F32, name="ps_q", tag=tag)
                for di in range(8):
                    nc.tensor.matmul(
                        ps, lhsT=wq_sb[:, di, dt_ * 128:(dt_ + 1) * 128],
                        rhs=xq_t[:, di, :], start=(di == 0), stop=(di == 7))
                nc.vector.tensor_scalar(
                    out=qT_sb[:, dt_, sc * 512:(sc + 1) * 512], in0=ps,
                    scalar1=bq_sb[:, dt_:dt_ + 1], scalar2=None, op0=ALU.add)

        def out_proj(so, pools, use_act=False):
            o_sb = opool.tile([128, D_MODEL], BF16, name="o_sb", tag="o")
            for oc in range(2):
                pool, tag = pools[(2 * so + oc) % len(pools)]
                ps = pool.tile([128, 512], F32, name="ps_o", tag=tag)
                for hp in range(2):
                    nc.tensor.matmul(
                        ps, lhsT=ctx_pair[:, hp, so * 128:(so + 1) * 128],
                        rhs=wo_sb[:, hp, oc * 512:(oc + 1) * 512],
                        start=(hp == 0), stop=(hp == 1))
                dst = o_sb[:, oc * 512:(oc + 1) * 512]
                if use_act and oc == 1:
                    nc.scalar.copy(dst, ps)
                else:
                    nc.vector.tensor_copy(out=dst, in_=ps)
            nc.sync.dma_start(out[so * 128:(so + 1) * 128, :], o_sb)

        # ---- attention step pipeline over all 4 (hp, qh) blocks ----
        blocks = [(0, 0), (1, 0), (0, 1), (1, 1)]
        steps = [(bi, st, qc)
                 for bi in range(4) for st in range(nt) for qc in (0, 1)]
        # denominator col-tile positions: (hh, qc) -> partition 32*j
        dnj = {(0, 0): 2, (1, 0): 3, (0, 1): 0, (1, 1): 1}
        state = {}
        pts = {}

        def bank_opens(bi):
            # Zero-weight full-bank matmuls open each accumulation bank:
            # clears has_written across all 128 partitions so the real
            # streams (start=False) overwrite-on-first-touch per element.
            st_ = state[bi] = {
                "ctx": [ps_ctx.tile([128, 512], F32, name=f"ctx{hh}",
                                    tag="ctx") for hh in range(2)],
                "dn": ps_dn.tile([128, 512], F32, name="dn", tag="dn"),
            }
            for bank in (st_["ctx"][0], st_["ctx"][1], st_["dn"]):
                nc.tensor.matmul(bank, lhsT=zeros_sb[:, 0:128], rhs=zeros_sb,
                                 start=True, stop=False, skip_group_check=True)

        def prepare(s):
            bi, st, qc = s
            hp, qh = blocks[bi]
            qoff = qh * 1024 + qc * 512
            sps = ps_sc.tile([128, 1024], F32, name="sps", tag="sc")
            for hh in range(2):
                nc.tensor.matmul(
                    sps[:, hh * 512:(hh + 1) * 512],
                    lhsT=kT_sb[64 * hh:64 * (hh + 1), hp,
                               st * 128:(st + 1) * 128],
                    rhs=qT_sb[64 * hh:64 * (hh + 1), hp, qoff:qoff + 512],
                    start=True, stop=True, tile_position=(64 * hh, 0))
            pt = ptpool.tile([128, 1024], BF16, name="pt", tag="pt")
            nc.scalar.activation(out=pt, in_=sps, func=AF.Exp,
                                 bias=mask_sb[:, st:st + 1], scale=0.125)
            pts[s] = pt

        def pv_dn(s):
            bi, st, qc = s
            hp, qh = blocks[bi]
            pt = pts.pop(s)
            ctx_ps = state[bi]["ctx"]
            dn_ps = state[bi]["dn"]
            for hh in range(2):
                nc.tensor.matmul(
                    ctx_ps[hh][64 * qc:64 * (qc + 1), :],
                    lhsT=v_tiles[st][:, 2 * hp + hh, :],
                    rhs=pt[:, hh * 512:(hh + 1) * 512],
                    start=(st == 0 and qc == 0),
                    stop=(st == nt - 1),
                    tile_position=(0, 64 * qc),
                    skip_group_check=True)
            for hh in range(2):
                j = dnj[(hh, qc)]
                nc.tensor.matmul(
                    dn_ps[32 * j:32 * j + 1, :],
                    lhsT=ones_bc[:, j:j + 1],
                    rhs=pt[:, hh * 512:(hh + 1) * 512],
                    start=(st == 0 and qc == 0 and hh == 0),
                    stop=(st == nt - 1),
                    tile_position=(0, 32 * j),
                    skip_group_check=True)

        def normalize_qc(bi, qc, cb=None):
            # per query-chunk normalize: reciprocal of this chunk's two
            # denominator rows, PE-broadcast to 64 partitions per head,
            # multiply into ctx_pair.  qc0 rows live at partitions 64-127
            # of the dn bank (j=2,3), qc1 at 0-63 (j=0,1).
            hp, qh = blocks[bi]
            ctx_ps = state[bi]["ctx"]
            dn_ps = state[bi]["dn"]
            # full-bank reciprocal (partition offset 0 required); rows of
            # the other chunk may hold partial sums nothing reads
            rp = rpool.tile([128, 512], F32, name="rp", tag="rp")
            nc.vector.reciprocal_approx_fast(out=rp, in_=dn_ps)
            rpb = rpool.tile([65, 512], BF16, name="rpb", tag="rpb")
            # hh0 -> partition 0, hh1 -> partition 64
            j0, j1 = dnj[(0, qc)], dnj[(1, qc)]
            nc.vector.tensor_copy(out=rpb[0:1, :],
                                  in_=rp[32 * j0:32 * j0 + 1, :])
            nc.vector.tensor_copy(out=rpb[64:65, :],
                                  in_=rp[32 * j1:32 * j1 + 1, :])
            r_ps = ps_mm.tile([128, 512], F32, name="r_ps", tag="mm")
            nc.tensor.matmul(r_ps[0:64, :], lhsT=ones_bc[0:1, :],
                             rhs=rpb[0:1, :], start=True, stop=True,
                             tile_position=(0, 0))
            nc.tensor.matmul(r_ps[64:128, :], lhsT=ones_bc[64:65, :],
                             rhs=rpb[64:65, :], start=True, stop=True,
                             tile_position=(64, 64))
            r_sb = rpool.tile([128, 512], F32, name="r_sb", tag="r_sb")
            nc.vector.tensor_copy(out=r_sb, in_=r_ps)
            qoff = qh * 1024 + qc * 512
            for hh in range(2):
                nc.vector.tensor_tensor(
                    ctx_pair[64 * hh:64 * (hh + 1), hp, qoff:qoff + 512],
                    ctx_ps[hh][64 * qc:64 * (qc + 1), :],
                    r_sb[64 * hh:64 * (hh + 1), :],
                    ALU.mult)
            if cb is not None:
                cb(qc)

        # ---- producer fillers: (step index) -> [thunks] ----
        # kT tile st must be resident before prepare((0,st,0)) fires at step
        # 2*st-3; v_tiles[st] before pv_dn((0,st,*)) at step 2*st.
        fillers = {}

        def add_filler(i, f):
            fillers.setdefault(i, []).append(f)

        add_filler(0, lambda: q_proj(1))
        for st in range(1, nt):
            add_filler(max(2 * st - 3, 0),
                       lambda st=st: k_proj(st * 128, 128))
            if st >= 1:
                add_filler(max(2 * st - 1, 1), lambda st=st: v_proj(st))
        add_filler(2 * nt - 1, lambda: nc.sync.dma_start(
            wo_sb, woT[:, :, :]))
        add_filler(2 * nt + 1, lambda: q_proj(2))
        add_filler(2 * nt + 3, lambda: q_proj(3))
        # qh0 output projections: so 0-3 need normalize(B) qc0, so 4-7 qc1;
        # placed at the C/D block starts to fill the bank_opens wait window
        for k in range(4):
            add_filler(4 * nt + 2 * k,
                       lambda k=k: out_proj(k, [(ps_mm, "mm")]))
            add_filler(6 * nt + 2 * k,
                       lambda k=k: out_proj(4 + k, [(ps_mm, "mm")]))

        def tail_cb(qc):
            for so in range(8 + 4 * qc, 12 + 4 * qc):
                pools = [(ps_mm, "mm"), (ps_dn, "dn")]
                if qc == 1:
                    pools.append((ps_ctx, "ctx"))
                out_proj(so, pools, use_act=True)

        # ---- prefix: HAM warmup, then minimal producers for (0,0,0) ----
        wm = ps_mm.tile([128, 512], F32, name="wm", tag="mm")
        for w in range(12):
            nc.tensor.matmul(wm, lhsT=zeros_sb[:, 0:128], rhs=zeros_sb,
                             start=(w == 0), stop=(w == 11))
        k_proj(0, 128, dn_free=True)
        q_proj(0, dn_free=True)
        nc.sync.dma_start(wv_sb, wvT[:, :, :])
        v_proj(0)
        bank_opens(0)

        # ---- the pipeline ----
        prepared = 0
        for i, s in enumerate(steps):
            for f in fillers.get(i, ()):
                f()
            while prepared < min(i + 3, len(steps)):
                prepare(steps[prepared])
                prepared += 1
            bi, st, qc = s
            if st == 0 and qc == 0 and bi > 0:
                # as late as possible: just before this block's first PV,
                # behind the already-queued next scores, so the wait on the
                # previous block's normalize reads doesn't block the queue
                bank_opens(bi)
            pv_dn(s)
            if st == nt - 1:  # per-chunk normalize on the last two steps
                normalize_qc(bi, qc, cb=tail_cb if bi == 3 else None)

    nc.finalize()
    return nc


_NC_CACHE: dict = {}
LAST_RESULTS = None


def _get_program(nt: int) -> bass.Bass:
    if nt not in _NC_CACHE:
        _NC_CACHE[nt] = build_program(nt)
    return _NC_CACHE[nt]


def make_in_maps(query, key_, value, mask, Wq, bq, Wk, Wv, Wo, nt):
    nk = nt * 128
    in_maps = []
    idxs = [np.flatnonzero(mask[b, 0, 0]) for b in range(BATCH)]
    for b in range(BATCH):
        n = len(idxs[b])
        xk = np.zeros((nk, D_MODEL), np.float32)
        xv = np.zeros((nk, D_MODEL), np.float32)
        xk[:n] = key_[b][idxs[b]]
        xv[:n] = value[b][idxs[b]]
        mbf = np.full(nt * 128, np.float32(MASK_BIAS), np.float32)
        mbf[:n] = 0.0
        mb = np.ascontiguousarray(mbf.reshape(nt, 128).T)
        # device layouts: xq [p, qc, t, 512], xk/xv [p, kt, t, 128],
        # w [p, t, 256], wo [p, hp, 1024] -- one contiguous DMA run per
        # partition per chunk
        xqTb = np.ascontiguousarray(
            query[b].T.astype(NPBF16).reshape(8, 128, 4, 512)
            .transpose(1, 2, 0, 3))
        xkTb = np.ascontiguousarray(
            xk.T.astype(NPBF16).reshape(8, 128, nt, 128)
            .transpose(1, 2, 0, 3))
        xvTb = np.ascontiguousarray(
            xv.T.astype(NPBF16).reshape(8, 128, nt, 128)
            .transpose(1, 2, 0, 3))
        for hg in range(4):
            sl = slice(hg * DO, (hg + 1) * DO)
            in_maps.append({
                "xqT": xqTb,
                "xkT": xkTb,
                "xvT": xvTb,
                "wqT": np.ascontiguousarray(
                    Wq[sl, :].T.astype(NPBF16).reshape(8, 128, DO)
                    .transpose(1, 0, 2)),
                "wkT": np.ascontiguousarray(
                    Wk[sl, :].T.astype(NPBF16).reshape(8, 128, DO)
                    .transpose(1, 0, 2)),
                "wvT": np.ascontiguousarray(
                    Wv[sl, :].T.astype(NPBF16).reshape(8, 128, DO)
                    .transpose(1, 0, 2)),
                "woT": np.ascontiguousarray(
                    Wo[:, sl].T.astype(NPBF16).reshape(2, 128, D_MODEL)
                    .transpose(1, 0, 2)),
                "bq2": np.ascontiguousarray(
                    bq[sl].reshape(2, 128).T.astype(np.float32)),
                "maskb": mb,
            })
    # reorder: core c = b*4 + hg
    return in_maps


def kernel(query, key_, value, mask, Wq, bq, Wk, bk, Wv, bv, Wo, bo):
    global LAST_RESULTS
    query = np.asarray(query, dtype=np.float32)
    key_ = np.asarray(key_, dtype=np.float32)
    value = np.asarray(value, dtype=np.float32)
    mask = np.asarray(mask)
    counts = [int(mask[b, 0, 0].sum()) for b in range(BATCH)]
    nt = max((max(counts) + 127) // 128, 1)
    nc = _get_program(nt)
    in_maps = make_in_maps(query, key_, value, mask,
                           np.asarray(Wq), np.asarray(bq), np.asarray(Wk),
                           np.asarray(Wv), np.asarray(Wo), nt)
    res = run_bass_kernel_spmd(nc, in_maps, list(range(N_CORES)))
    LAST_RESULTS = res
    corr = (np.asarray(bv, dtype=np.float32) @ np.asarray(Wo, dtype=np.float32).T
            + np.asarray(bo, dtype=np.float32))
    out = np.zeros((BATCH, S, D_MODEL), np.float32)
    for c in range(N_CORES):
        out[c // 4] += np.asarray(res.results[c]["out"], dtype=np.float32)
    out += corr[None, None, :]
    return out
